# revision 11
# baseline (speedup 1.0000x reference)
"""Masked dot-product attention on 8 Trainium2 NeuronCores (Bass/Tile).

Problem: queries/keys/values [32, 1024, 128] f32, valid_lens [32] i32.
  out = softmax(mask(Q K^T / sqrt(128))) V        (key-padding prefix mask)

Strategy (batch-parallel, 4 batches per core, one SPMD program):
  * Host pre-transposes Q and K per batch to [D=128, 1024] (bf16) so the
    contraction dim D sits on SBUF partitions; no on-device transposes.
  * Scores are computed transposed: S^T[k, q] = (K^T chunk).T @ Q^T with k
    in chunks of 128 partitions, accumulating in f32 PSUM.
  * The prefix key mask is per-PARTITION in this layout, so it folds into
    the exp for free: ACT computes exp(S^T * 1/sqrt(D) + bias) with
    bias[k] in {0, -1e6}; masked rows become exactly 0. Probs are bf16.
  * out^T[v, q] += V_chunk-as-lhsT @ expS^T accumulates in PSUM across
    k chunks (V is loaded chunk-major, no transpose needed).
  * denominator[q]: expS^T chunks are reduced with a binary-counter add
    tree on DVE (bf16, 2x mode), so the PE does only ONE two-matmul
    partition-reduction per batch into a [2, 512] PSUM bank (lhsT
    columns [1,0] / [0,1]), deferred three chunks so the tree adds never
    stall the in-order PE queue. The final slot keeps pair-level groups
    instead, so the kernel tail is not lengthened by the fold chain.
  * out^T and sums are DMA'd back in f32; the host divides and
    transposes while gathering (0.003% of the FLOPs).
  * All matmul operands are bf16 (1 cycle/row, FWL weight loads); PSUM
    accumulation stays f32, final output is f32. End-to-end rel err vs
    the f32 reference is ~3e-3 (tolerance 2e-2).

Startup/teardown engineering (the steady state is ACT-exp-bound at
~1.1us/chunk, so wins come from the edges):
  * Input DMAs are split across BOTH HWDGE rings: kt/qt stream on the
    Sync ring, mask/V/consts on the Scalar ring, interleaved per batch
    so vp[b] lands right after qt[b].
  * Every input tile is fully resident (per-slot tags, bufs=1), so no
    DMA issue ever waits on a buffer-reuse semaphore; those waits would
    head-of-line-block the issuing engine's queue.
  * PE + ACT warmup runs on a memset tile with NO DMA dependency: dummy
    matmuls fill the initial DMA window so the PE's HAM clock-gate
    reaches 8/8 before real work, and a dummy exp triggers the one-time
    ~1.3us ACT_TABLE_LOAD immediately after the framework preamble.
  * Epilogue PSUM->SBUF copies run on DVE only, emitted before the
    batch-final tree adds so the accumulator bank frees early.
  * Only two tile pools (one SBUF, one PSUM): pool boundaries cost
    cross-engine barriers in the teardown.

Static masked-chunk skipping: batch b only needs ceil(valid_lens[b]/128)
key chunks; the rest contribute exactly 0. Batches are assigned to the 4
per-core slots by descending need (sorted, slot-major), so slot j's
compile-time chunk count is max over its 8 batches. The SPMD program is
specialized to that profile at kernel build time.

The chunk loop is software-pipelined: chunk c+1's score matmuls are
emitted before chunk c's AV/sums matmuls so the PE produces the next
exp's input first and ACT never starves.
"""

import math

import ml_dtypes
import numpy as np

import concourse.bacc as bacc
import concourse.bass as bass
import concourse.mybir as mybir
import concourse.tile as tile
from concourse.bass_utils import run_bass_kernel_spmd

B, Q, K, D = 32, 1024, 1024, 128
N_CORES = 8
BPC = B // N_CORES  # batches per core
PART = 128          # partition size / key chunk size
NCHUNK = K // PART
MASK_BIAS = -1.0e6
INV_SQRT_D = 1.0 / math.sqrt(D)
F32 = mybir.dt.float32
BF16 = mybir.dt.bfloat16
NP_BF16 = ml_dtypes.bfloat16
N_WARM_MM = 5       # dummy PE matmuls (512 cols each) during the DMA wait
P_BUFS = 12         # probs-tile ring size

_NC_CACHE: dict = {}


def build_nc(profile: tuple) -> bass.Bass:
    """Build the SPMD Bass program for a per-slot chunk-count profile."""
    nc = bacc.Bacc()
    qt = nc.declare_dram_parameter("qt", [BPC, PART, Q], BF16, isOutput=False)
    kt = nc.declare_dram_parameter("kt", [BPC, PART, K], BF16, isOutput=False)
    vp = nc.declare_dram_parameter("vp", [BPC, PART, K], BF16, isOutput=False)
    mb = nc.declare_dram_parameter("mb", [PART, BPC * NCHUNK], F32, isOutput=False)
    cst = nc.declare_dram_parameter("cst", [PART, 4], BF16, isOutput=False)
    out = nc.declare_dram_parameter("out", [BPC, PART, Q], F32, isOutput=True)
    sums_out = nc.declare_dram_parameter("sums", [BPC, 2, 512], F32, isOutput=True)

    with tile.TileContext(nc) as tc:
        with (
            tc.tile_pool(name="sb", bufs=1) as sb,
            tc.tile_pool(name="ps", bufs=1, space="PSUM") as ps,
        ):
            # Warmup with no DMA dependency: memset a tile, then hammer the
            # PE with dummy matmuls (HAM warm) and ACT with a dummy exp
            # (one-time exp table load) while the first inputs stream in.
            # The warm/filler PSUM bank is never read by any engine.
            warm_sb = sb.tile([PART, 512], BF16, tag="warm")
            nc.vector.memset(warm_sb, 1.0)
            warm_ps = ps.tile([PART, 512], F32, tag="fill", bufs=1)
            for _ in range(N_WARM_MM):
                nc.tensor.matmul(
                    warm_ps, warm_sb[:, 0:PART], warm_sb, start=True, stop=True
                )

            def filler_mm(i):
                # Dep-free dummy matmul slotted where the in-order PE queue
                # would otherwise micro-idle waiting on the current exp.
                # Keeps the PE's HAM activity window busy so the clock-gate
                # stays at 8/8; an idle-throttled PE (427ns vs 216ns per
                # matmul) cannot keep ahead of ACT and starves the exps.
                f_ps = ps.tile([PART, PART], F32, tag="fill", bufs=1,
                               name=f"fill_{i}")
                nc.tensor.matmul(
                    f_ps, warm_sb[:, 0:PART], warm_sb[:, 0:PART],
                    start=True, stop=True,
                )
            # Input streams. The startup-critical batch-0 loads ride the
            # otherwise-idle Scalar ring: qt0 as two half-DMAs (the first
            # score matmul only needs the first half), then the mask, vp0,
            # the dummy exp (which pulls in the one-time ACT_TABLE_LOAD),
            # and cst. Everything else streams on the Sync ring, vp[b]
            # right after qt[b] so V lands before its batch's AV matmuls.
            ins_sb = []
            for b in range(BPC):
                cap = profile[b]
                kcols = cap * PART
                qt_sb = sb.tile([PART, Q], BF16, tag=f"qt{b}", name=f"qt{b}")
                kt_sb = sb.tile([PART, kcols], BF16, tag=f"kt{b}", name=f"kt{b}")
                vp_sb = sb.tile([PART, kcols], BF16, tag=f"vp{b}", name=f"vp{b}")
                ins_sb.append((qt_sb, kt_sb, vp_sb))
                nc.sync.dma_start(out=kt_sb, in_=kt[b][:, :kcols])
                if b == 0:
                    nc.scalar.dma_start(out=qt_sb[:, 0:512], in_=qt[b][:, 0:512])
                    nc.scalar.dma_start(
                        out=qt_sb[:, 512:1024], in_=qt[b][:, 512:1024]
                    )
                    mb_sb = sb.tile([PART, BPC * NCHUNK], F32, tag="mb")
                    nc.scalar.dma_start(out=mb_sb, in_=mb[:, :])
                    nc.scalar.dma_start(out=vp_sb, in_=vp[b][:, :kcols])
                    warm_act = sb.tile([PART, 1], F32, tag="warm_act")
                    nc.scalar.activation(
                        warm_act,
                        warm_sb[:, 0:1],
                        mybir.ActivationFunctionType.Exp,
                        scale=0.0,
                    )
                    cst_sb = sb.tile([PART, 4], BF16, tag="cst")
                    nc.scalar.dma_start(out=cst_sb, in_=cst[:, :])
                else:
                    nc.sync.dma_start(out=qt_sb, in_=qt[b])
                    nc.sync.dma_start(out=vp_sb, in_=vp[b][:, :kcols])

            # Flat chunk stream across batches with 2-deep score lookahead:
            # the in-order PE queue must see the next chunks' score matmuls
            # BEFORE a batch-boundary AV matmul that may stall on the PSUM
            # accumulator release.
            stream = [(b, c) for b in range(BPC) for c in range(profile[b])]

            def s_mms(b, c):
                qt_sb, kt_sb, _ = ins_sb[b]
                s_ps = ps.tile([PART, Q], F32, tag="s", bufs=2, name=f"s_b{b}c{c}")
                kw = kt_sb[:, c * PART:(c + 1) * PART]
                for h in range(2):
                    nc.tensor.matmul(
                        s_ps[:, h * 512:(h + 1) * 512],
                        kw,
                        qt_sb[:, h * 512:(h + 1) * 512],
                        start=True,
                        stop=True,
                    )
                return s_ps

            def p_tile(nm):
                return sb.tile([PART, Q], BF16, tag="p", bufs=P_BUFS, name=nm)

            def sums_mms(sums_ps, rhs_t, st, sp):
                # Rows [sum of h0 cols; sum of h1 cols] into one PSUM bank:
                # lhsT columns are [1,0] (cst cols 0:2) and [0,1] (cols 2:4).
                nc.tensor.matmul(
                    sums_ps[0:2, 0:512],
                    cst_sb[:, 0:2],
                    rhs_t[:, 0:512],
                    start=st,
                    stop=False,
                )
                nc.tensor.matmul(
                    sums_ps[0:2, 0:512],
                    cst_sb[:, 2:4],
                    rhs_t[:, 512:1024],
                    start=False,
                    stop=sp,
                )

            def sums_epilogue(b, sums_ps):
                sums_sb = sb.tile(
                    [2, 512], F32, tag="sums_sb", bufs=2, name=f"sums_sb{b}"
                )
                nc.vector.tensor_copy(sums_sb, sums_ps)
                nc.sync.dma_start(out=sums_out[b], in_=sums_sb)

            # pend entries: (due_i, sums_ps, rhs, st, sp, b_if_final_group)
            pend = []

            def flush_pend(i):
                keep = []
                for e in pend:
                    if e[0] <= i:
                        _, ps_t, rhs_t, st, sp, eb = e
                        sums_mms(ps_t, rhs_t, st, sp)
                        if eb is not None:
                            sums_epilogue(eb, ps_t)
                    else:
                        keep.append(e)
                pend[:] = keep

            s_tiles = {}
            for j in range(min(2, len(stream))):
                s_tiles[stream[j]] = s_mms(*stream[j])
            acc = {}
            tree = {b: [] for b in range(BPC)}  # binary-counter add levels
            prev_p = {}
            for i, (b, c) in enumerate(stream):
                cap = profile[b]
                final_slot = b == BPC - 1
                if c == 0:
                    out_ps = ps.tile(
                        [PART, Q], F32, tag="out", bufs=1, name=f"out_b{b}"
                    )
                    sums_ps = ps.tile(
                        [2, 512], F32, tag="sums", bufs=1, name=f"sums_b{b}"
                    )
                    acc[b] = (out_ps, sums_ps)
                out_ps, sums_ps = acc[b]
                p_sb = p_tile(f"p_{i}")
                nc.scalar.activation(
                    p_sb,
                    s_tiles.pop((b, c)),
                    mybir.ActivationFunctionType.Exp,
                    bias=mb_sb[:, b * NCHUNK + c:b * NCHUNK + c + 1],
                    scale=INV_SQRT_D,
                )
                if i + 2 < len(stream):
                    s_tiles[stream[i + 2]] = s_mms(*stream[i + 2])
                flush_pend(i)
                vw = ins_sb[b][2][:, c * PART:(c + 1) * PART]
                first, last = c == 0, c == cap - 1
                for h in range(2):
                    nc.tensor.matmul(
                        out_ps[:, h * 512:(h + 1) * 512],
                        vw,
                        p_sb[:, h * 512:(h + 1) * 512],
                        start=first,
                        stop=last,
                    )
                filler_mm(i)
                if last:
                    # Epilogue out-copies first: the accumulator bank frees
                    # before the batch-final DVE adds run.
                    outn = sb.tile([PART, Q], F32, tag="outn", bufs=3, name=f"outn{b}")
                    nc.vector.tensor_copy(outn[:, 0:512], out_ps[:, 0:512])
                    nc.sync.dma_start(out=out[b][:, 0:512], in_=outn[:, 0:512])
                    nc.vector.tensor_copy(outn[:, 512:1024], out_ps[:, 512:1024])
                    nc.sync.dma_start(
                        out=out[b][:, 512:1024], in_=outn[:, 512:1024]
                    )
                if not final_slot:
                    if not last:
                        # Binary-counter add tree: carry-add pending levels.
                        cur = p_sb
                        k = 0
                        lv = tree[b]
                        while k < len(lv) and lv[k] is not None:
                            t = p_tile(f"t{k}_{i}")
                            nc.gpsimd.tensor_add(t, lv[k], cur)
                            lv[k] = None
                            cur = t
                            k += 1
                        if k == len(lv):
                            lv.append(cur)
                        else:
                            lv[k] = cur
                    else:
                        # Fold everything into one group; sums matmuls are
                        # deferred 3 chunks so the fold chain never stalls
                        # the PE queue.
                        cur = p_sb
                        for t in tree[b]:
                            if t is not None:
                                t2 = p_tile(f"fold_{i}")
                                nc.gpsimd.tensor_add(t2, t, cur)
                                cur = t2
                        pend.append((i + 3, sums_ps, cur, True, True, b))
                else:
                    # Final slot: pair-level groups keep the tail short.
                    if cap == 1 or (last and c % 2 == 0):
                        pend.append((i + 1, sums_ps, p_sb, c == 0, True,
                                     b if last else None))
                    elif c % 2 == 0:
                        prev_p[b] = p_sb
                    else:
                        pair_sb = p_tile(f"pair_{i}")
                        nc.gpsimd.tensor_add(pair_sb, prev_p.pop(b), p_sb)
                        pend.append((i + 1, sums_ps, pair_sb, c == 1,
                                     c >= cap - 2, b if c >= cap - 2 else None))

            flush_pend(len(stream) + 3)

    nc.compile()
    return nc


def plan(valid_lens: np.ndarray):
    """Assign batches to (core, slot) and derive the chunk-count profile.

    Sorting by descending need and slicing slot-major minimizes the sum of
    per-slot maxima, which is the per-core static work.
    """
    need = np.minimum((valid_lens.astype(np.int64) + PART - 1) // PART, NCHUNK)
    need = np.maximum(need, 1)
    order = np.argsort(-need, kind="stable")
    perm = order.reshape(BPC, N_CORES)  # perm[slot, core] = batch index
    # Process the smallest slot first: its input DMAs are the ones compute
    # must wait for at startup; the bigger slots' loads overlap compute.
    rot = np.argsort([int(need[perm[s]].max()) for s in range(BPC)], kind="stable")
    rot = np.concatenate([rot[:1], rot[1:][::-1]])  # smallest, then descending
    perm = perm[rot]
    profile = tuple(int(need[perm[s]].max()) for s in range(BPC))
    return perm, profile


def host_prep(q, k, v, lens):
    """Shard + lay out inputs for the 8 cores. Returns (perm, profile, in_maps)."""
    perm, profile = plan(lens)

    # Vectorized host layout prep: obi[core, slot] = batch index.
    obi = perm.T  # [N_CORES, BPC]
    qt_all = np.ascontiguousarray(
        q[obi].transpose(0, 1, 3, 2).astype(NP_BF16)
    )  # [8,4,128,1024]
    kt_all = np.ascontiguousarray(k[obi].transpose(0, 1, 3, 2).astype(NP_BF16))
    # v chunk-major: vp[p, c*128 + d] = v[c*128 + p, d]
    vp_all = np.ascontiguousarray(
        v[obi]
        .reshape(N_CORES, BPC, NCHUNK, PART, D)
        .transpose(0, 1, 3, 2, 4)
        .reshape(N_CORES, BPC, PART, K)
        .astype(NP_BF16)
    )
    # bias[p, slot*8 + c] = 0 if (c*128+p) < L else -1e6
    valid = np.arange(K)[None, None, :] < lens[obi][:, :, None]  # [8,4,1024]
    mb_all = np.where(
        valid.reshape(N_CORES, BPC, NCHUNK, PART).transpose(0, 2, 3, 1), 0.0, MASK_BIAS
    ).astype(np.float32)  # [8, NCHUNK, PART, BPC] -> need [8, PART, BPC*NCHUNK]
    mb_all = np.ascontiguousarray(
        mb_all.transpose(0, 2, 3, 1).reshape(N_CORES, PART, BPC * NCHUNK)
    )
    # cst columns [1,0] and [0,1]: the sums-matmul weight pairs.
    cstv = np.zeros((PART, 4), NP_BF16)
    cstv[:, 0] = 1
    cstv[:, 3] = 1

    in_maps = [
        {
            "qt": qt_all[core],
            "kt": kt_all[core],
            "vp": vp_all[core],
            "mb": mb_all[core],
            "cst": cstv,
        }
        for core in range(N_CORES)
    ]
    return perm, profile, in_maps


def kernel(queries, keys, values, valid_lens):
    q = np.ascontiguousarray(np.asarray(queries, dtype=np.float32))
    k = np.ascontiguousarray(np.asarray(keys, dtype=np.float32))
    v = np.ascontiguousarray(np.asarray(values, dtype=np.float32))
    lens = np.asarray(valid_lens).astype(np.int64).reshape(B)

    perm, profile, in_maps = host_prep(q, k, v, lens)

    if profile not in _NC_CACHE:
        _NC_CACHE[profile] = build_nc(profile)
    nc = _NC_CACHE[profile]

    res = run_bass_kernel_spmd(nc, in_maps, list(range(N_CORES)))

    out = np.empty((B, Q, D), np.float32)
    for core in range(N_CORES):
        core_out = res.results[core]["out"]    # [BPC, 128(v), 1024(q)]
        core_sums = res.results[core]["sums"]  # [BPC, 2, 512] -> flat [1024(q)]
        for slot in range(BPC):
            bidx = int(perm[slot, core])
            sums_flat = core_sums[slot].reshape(Q)
            out[bidx] = (core_out[slot] / sums_flat[None, :]).T
    return out


# revision 13
# speedup vs baseline: 1.2408x; 1.2408x over previous
"""Masked dot-product attention on 8 Trainium2 NeuronCores (Bass/Tile).

Problem: queries/keys/values [32, 1024, 128] f32, valid_lens [32] i32.
  out = softmax(mask(Q K^T / sqrt(128))) V        (key-padding prefix mask)

Strategy (batch-parallel, 4 batches per core, one SPMD program):
  * Host pre-transposes Q and K per batch to [D=128, 1024] (bf16) so the
    contraction dim D sits on SBUF partitions; no on-device transposes.
  * Scores are computed transposed: S^T[k, q] = (K^T chunk).T @ Q^T with k
    in chunks of 128 partitions, accumulating in f32 PSUM.
  * The prefix key mask is per-PARTITION in this layout, so it folds into
    the exp for free: ACT computes exp(S^T * 1/sqrt(D) + bias) with
    bias[k] in {0, -1e6}; masked rows become exactly 0. Probs are bf16.
  * out^T[v, q] += V_chunk-as-lhsT @ expS^T accumulates in PSUM across
    k chunks (V is loaded chunk-major, no transpose needed).
  * denominator[q]: expS^T chunks are reduced with a binary-counter add
    tree on DVE (bf16, 2x mode), so the PE does only ONE two-matmul
    partition-reduction per batch into a [2, 512] PSUM bank (lhsT
    columns [1,0] / [0,1]), deferred three chunks so the tree adds never
    stall the in-order PE queue. The final slot keeps pair-level groups
    instead, so the kernel tail is not lengthened by the fold chain.
  * out^T and sums are DMA'd back in f32; the host divides and
    transposes while gathering (0.003% of the FLOPs).
  * All matmul operands are bf16 (1 cycle/row, FWL weight loads); PSUM
    accumulation stays f32, final output is f32. End-to-end rel err vs
    the f32 reference is ~3e-3 (tolerance 2e-2).

Startup/teardown engineering (the steady state is ACT-exp-bound at
~1.1us/chunk, so wins come from the edges):
  * Input DMAs are split across BOTH HWDGE rings: kt/qt stream on the
    Sync ring, mask/V/consts on the Scalar ring, interleaved per batch
    so vp[b] lands right after qt[b].
  * Every input tile is fully resident (per-slot tags, bufs=1), so no
    DMA issue ever waits on a buffer-reuse semaphore; those waits would
    head-of-line-block the issuing engine's queue.
  * PE + ACT warmup runs on a memset tile with NO DMA dependency: dummy
    matmuls fill the initial DMA window so the PE's HAM clock-gate
    reaches 8/8 before real work, and a dummy exp triggers the one-time
    ~1.3us ACT_TABLE_LOAD immediately after the framework preamble.
  * Epilogue PSUM->SBUF copies run on DVE only, emitted before the
    batch-final tree adds so the accumulator bank frees early.
  * Only two tile pools (one SBUF, one PSUM): pool boundaries cost
    cross-engine barriers in the teardown.

Static masked-chunk skipping: batch b only needs ceil(valid_lens[b]/128)
key chunks; the rest contribute exactly 0. Batches are assigned to the 4
per-core slots by descending need (sorted, slot-major), so slot j's
compile-time chunk count is max over its 8 batches. The SPMD program is
specialized to that profile at kernel build time.

The chunk loop is software-pipelined: chunk c+1's score matmuls are
emitted before chunk c's AV/sums matmuls so the PE produces the next
exp's input first and ACT never starves.
"""

import math

import ml_dtypes
import numpy as np

import concourse.bacc as bacc
import concourse.bass as bass
import concourse.mybir as mybir
import concourse.tile as tile
from concourse.bass_utils import run_bass_kernel_spmd

B, Q, K, D = 32, 1024, 1024, 128
N_CORES = 8
BPC = B // N_CORES  # batches per core
PART = 128          # partition size / key chunk size
NCHUNK = K // PART
MASK_BIAS = -1.0e6
INV_SQRT_D = 1.0 / math.sqrt(D)
F32 = mybir.dt.float32
BF16 = mybir.dt.bfloat16
NP_BF16 = ml_dtypes.bfloat16
N_WARM_MM = 5       # dummy PE matmuls (512 cols each) during the DMA wait
P_BUFS = 12         # probs-tile ring size

_NC_CACHE: dict = {}


def build_nc(profile: tuple) -> bass.Bass:
    """Build the SPMD Bass program for a per-slot chunk-count profile."""
    nc = bacc.Bacc()
    qt = nc.declare_dram_parameter("qt", [BPC, PART, Q], BF16, isOutput=False)
    kt = nc.declare_dram_parameter("kt", [BPC, PART, K], BF16, isOutput=False)
    vp = nc.declare_dram_parameter("vp", [BPC, PART, K], BF16, isOutput=False)
    mb = nc.declare_dram_parameter("mb", [PART, BPC * NCHUNK], F32, isOutput=False)
    cst = nc.declare_dram_parameter("cst", [PART, 4], BF16, isOutput=False)
    out = nc.declare_dram_parameter("out", [BPC, PART, Q], BF16, isOutput=True)
    sums_out = nc.declare_dram_parameter("sums", [BPC, 2, 512], F32, isOutput=True)

    with tile.TileContext(nc) as tc:
        with (
            tc.tile_pool(name="sb", bufs=1) as sb,
            tc.tile_pool(name="ps", bufs=1, space="PSUM") as ps,
        ):
            # Warmup with no DMA dependency: memset a tile, then hammer the
            # PE with dummy matmuls (HAM warm) and ACT with a dummy exp
            # (one-time exp table load) while the first inputs stream in.
            # The warm/filler PSUM bank is never read by any engine.
            warm_sb = sb.tile([PART, 512], BF16, tag="warm")
            nc.vector.memset(warm_sb, 1.0)
            warm_ps = ps.tile([PART, 512], F32, tag="fill", bufs=1)
            for _ in range(N_WARM_MM):
                nc.tensor.matmul(
                    warm_ps, warm_sb[:, 0:PART], warm_sb, start=True, stop=True
                )

            def filler_mm(i):
                # Dep-free dummy matmul slotted where the in-order PE queue
                # would otherwise micro-idle waiting on the current exp.
                # Keeps the PE's HAM activity window busy so the clock-gate
                # stays at 8/8; an idle-throttled PE (427ns vs 216ns per
                # matmul) cannot keep ahead of ACT and starves the exps.
                f_ps = ps.tile([PART, PART], F32, tag="fill", bufs=1,
                               name=f"fill_{i}")
                nc.tensor.matmul(
                    f_ps, warm_sb[:, 0:PART], warm_sb[:, 0:PART],
                    start=True, stop=True,
                )
            # Input streams. The startup-critical batch-0 loads ride the
            # otherwise-idle Scalar ring: qt0 as two half-DMAs (the first
            # score matmul only needs the first half), then the mask, vp0,
            # the dummy exp (which pulls in the one-time ACT_TABLE_LOAD),
            # and cst. Everything else streams on the Sync ring, vp[b]
            # right after qt[b] so V lands before its batch's AV matmuls.
            ins_sb = []
            for b in range(BPC):
                cap = profile[b]
                kcols = cap * PART
                qt_sb = sb.tile([PART, Q], BF16, tag=f"qt{b}", name=f"qt{b}")
                kt_sb = sb.tile([PART, kcols], BF16, tag=f"kt{b}", name=f"kt{b}")
                vp_sb = sb.tile([PART, kcols], BF16, tag=f"vp{b}", name=f"vp{b}")
                ins_sb.append((qt_sb, kt_sb, vp_sb))
                nc.sync.dma_start(out=kt_sb, in_=kt[b][:, :kcols])
                if b == 0:
                    nc.scalar.dma_start(out=qt_sb[:, 0:512], in_=qt[b][:, 0:512])
                    nc.scalar.dma_start(
                        out=qt_sb[:, 512:1024], in_=qt[b][:, 512:1024]
                    )
                    mb_sb = sb.tile([PART, BPC * NCHUNK], F32, tag="mb")
                    nc.scalar.dma_start(out=mb_sb, in_=mb[:, :])
                    nc.scalar.dma_start(out=vp_sb, in_=vp[b][:, :kcols])
                    warm_act = sb.tile([PART, 1], F32, tag="warm_act")
                    nc.scalar.activation(
                        warm_act,
                        warm_sb[:, 0:1],
                        mybir.ActivationFunctionType.Exp,
                        scale=0.0,
                    )
                    cst_sb = sb.tile([PART, 4], BF16, tag="cst")
                    nc.scalar.dma_start(out=cst_sb, in_=cst[:, :])
                else:
                    nc.sync.dma_start(out=qt_sb, in_=qt[b])
                    nc.sync.dma_start(out=vp_sb, in_=vp[b][:, :kcols])

            # Flat chunk stream across batches with 2-deep score lookahead:
            # the in-order PE queue must see the next chunks' score matmuls
            # BEFORE a batch-boundary AV matmul that may stall on the PSUM
            # accumulator release.
            stream = [(b, c) for b in range(BPC) for c in range(profile[b])]

            def s_mms(b, c):
                qt_sb, kt_sb, _ = ins_sb[b]
                s_ps = ps.tile([PART, Q], F32, tag="s", bufs=2, name=f"s_b{b}c{c}")
                kw = kt_sb[:, c * PART:(c + 1) * PART]
                for h in range(2):
                    nc.tensor.matmul(
                        s_ps[:, h * 512:(h + 1) * 512],
                        kw,
                        qt_sb[:, h * 512:(h + 1) * 512],
                        start=True,
                        stop=True,
                    )
                return s_ps

            def p_tile(nm):
                return sb.tile([PART, Q], BF16, tag="p", bufs=P_BUFS, name=nm)

            def sums_mms(sums_ps, rhs_t, st, sp):
                # Rows [sum of h0 cols; sum of h1 cols] into one PSUM bank:
                # lhsT columns are [1,0] (cst cols 0:2) and [0,1] (cols 2:4).
                nc.tensor.matmul(
                    sums_ps[0:2, 0:512],
                    cst_sb[:, 0:2],
                    rhs_t[:, 0:512],
                    start=st,
                    stop=False,
                )
                nc.tensor.matmul(
                    sums_ps[0:2, 0:512],
                    cst_sb[:, 2:4],
                    rhs_t[:, 512:1024],
                    start=False,
                    stop=sp,
                )

            def sums_epilogue(b, sums_ps):
                sums_sb = sb.tile(
                    [2, 512], F32, tag="sums_sb", bufs=2, name=f"sums_sb{b}"
                )
                nc.vector.tensor_copy(sums_sb, sums_ps)
                nc.sync.dma_start(out=sums_out[b], in_=sums_sb)

            # pend entries: (due_i, sums_ps, rhs, st, sp, b_if_final_group)
            pend = []

            def flush_pend(i):
                keep = []
                for e in pend:
                    if e[0] <= i:
                        _, ps_t, rhs_t, st, sp, eb = e
                        sums_mms(ps_t, rhs_t, st, sp)
                        if eb is not None:
                            sums_epilogue(eb, ps_t)
                    else:
                        keep.append(e)
                pend[:] = keep

            s_tiles = {}
            for j in range(min(2, len(stream))):
                s_tiles[stream[j]] = s_mms(*stream[j])
            acc = {}
            tree = {b: [] for b in range(BPC)}  # binary-counter add levels
            prev_p = {}
            for i, (b, c) in enumerate(stream):
                cap = profile[b]
                final_slot = b == BPC - 1
                if c == 0:
                    out_ps = ps.tile(
                        [PART, Q], F32, tag="out", bufs=1, name=f"out_b{b}"
                    )
                    sums_ps = ps.tile(
                        [2, 512], F32, tag="sums", bufs=1, name=f"sums_b{b}"
                    )
                    acc[b] = (out_ps, sums_ps)
                out_ps, sums_ps = acc[b]
                p_sb = p_tile(f"p_{i}")
                nc.scalar.activation(
                    p_sb,
                    s_tiles.pop((b, c)),
                    mybir.ActivationFunctionType.Exp,
                    bias=mb_sb[:, b * NCHUNK + c:b * NCHUNK + c + 1],
                    scale=INV_SQRT_D,
                )
                if i + 2 < len(stream):
                    s_tiles[stream[i + 2]] = s_mms(*stream[i + 2])
                flush_pend(i)
                vw = ins_sb[b][2][:, c * PART:(c + 1) * PART]
                first, last = c == 0, c == cap - 1
                for h in range(2):
                    nc.tensor.matmul(
                        out_ps[:, h * 512:(h + 1) * 512],
                        vw,
                        p_sb[:, h * 512:(h + 1) * 512],
                        start=first,
                        stop=last,
                    )
                filler_mm(i)
                if last:
                    # Epilogue out-copies first: the accumulator bank frees
                    # before the batch-final DVE adds run.
                    outn = sb.tile([PART, Q], BF16, tag="outn", bufs=3, name=f"outn{b}")
                    nc.vector.tensor_copy(outn[:, 0:512], out_ps[:, 0:512])
                    nc.sync.dma_start(out=out[b][:, 0:512], in_=outn[:, 0:512])
                    nc.vector.tensor_copy(outn[:, 512:1024], out_ps[:, 512:1024])
                    nc.sync.dma_start(
                        out=out[b][:, 512:1024], in_=outn[:, 512:1024]
                    )
                if not final_slot:
                    if not last:
                        # Binary-counter add tree: carry-add pending levels.
                        cur = p_sb
                        k = 0
                        lv = tree[b]
                        while k < len(lv) and lv[k] is not None:
                            t = p_tile(f"t{k}_{i}")
                            nc.vector.tensor_add(t, lv[k], cur)
                            lv[k] = None
                            cur = t
                            k += 1
                        if k == len(lv):
                            lv.append(cur)
                        else:
                            lv[k] = cur
                    else:
                        # Fold everything into one group; sums matmuls are
                        # deferred 3 chunks so the fold chain never stalls
                        # the PE queue.
                        cur = p_sb
                        for t in tree[b]:
                            if t is not None:
                                t2 = p_tile(f"fold_{i}")
                                nc.vector.tensor_add(t2, t, cur)
                                cur = t2
                        pend.append((i + 3, sums_ps, cur, True, True, b))
                else:
                    # Final slot: pair-level groups keep the tail short.
                    if cap == 1 or (last and c % 2 == 0):
                        pend.append((i + 1, sums_ps, p_sb, c == 0, True,
                                     b if last else None))
                    elif c % 2 == 0:
                        prev_p[b] = p_sb
                    else:
                        pair_sb = p_tile(f"pair_{i}")
                        nc.vector.tensor_add(pair_sb, prev_p.pop(b), p_sb)
                        pend.append((i + 1, sums_ps, pair_sb, c == 1,
                                     c >= cap - 2, b if c >= cap - 2 else None))

            flush_pend(len(stream) + 3)

    nc.compile()
    return nc


def plan(valid_lens: np.ndarray):
    """Assign batches to (core, slot) and derive the chunk-count profile.

    Sorting by descending need and slicing slot-major minimizes the sum of
    per-slot maxima, which is the per-core static work.
    """
    need = np.minimum((valid_lens.astype(np.int64) + PART - 1) // PART, NCHUNK)
    need = np.maximum(need, 1)
    order = np.argsort(-need, kind="stable")
    perm = order.reshape(BPC, N_CORES)  # perm[slot, core] = batch index
    # Process the smallest slot first: its input DMAs are the ones compute
    # must wait for at startup; the bigger slots' loads overlap compute.
    rot = np.argsort([int(need[perm[s]].max()) for s in range(BPC)], kind="stable")
    rot = np.concatenate([rot[:1], rot[1:][::-1]])  # smallest, then descending
    perm = perm[rot]
    profile = tuple(int(need[perm[s]].max()) for s in range(BPC))
    return perm, profile


def host_prep(q, k, v, lens):
    """Shard + lay out inputs for the 8 cores. Returns (perm, profile, in_maps)."""
    perm, profile = plan(lens)

    # Vectorized host layout prep: obi[core, slot] = batch index.
    obi = perm.T  # [N_CORES, BPC]
    qt_all = np.ascontiguousarray(
        q[obi].transpose(0, 1, 3, 2).astype(NP_BF16)
    )  # [8,4,128,1024]
    kt_all = np.ascontiguousarray(k[obi].transpose(0, 1, 3, 2).astype(NP_BF16))
    # v chunk-major: vp[p, c*128 + d] = v[c*128 + p, d]
    vp_all = np.ascontiguousarray(
        v[obi]
        .reshape(N_CORES, BPC, NCHUNK, PART, D)
        .transpose(0, 1, 3, 2, 4)
        .reshape(N_CORES, BPC, PART, K)
        .astype(NP_BF16)
    )
    # bias[p, slot*8 + c] = 0 if (c*128+p) < L else -1e6
    valid = np.arange(K)[None, None, :] < lens[obi][:, :, None]  # [8,4,1024]
    mb_all = np.where(
        valid.reshape(N_CORES, BPC, NCHUNK, PART).transpose(0, 2, 3, 1), 0.0, MASK_BIAS
    ).astype(np.float32)  # [8, NCHUNK, PART, BPC] -> need [8, PART, BPC*NCHUNK]
    mb_all = np.ascontiguousarray(
        mb_all.transpose(0, 2, 3, 1).reshape(N_CORES, PART, BPC * NCHUNK)
    )
    # cst columns [1,0] and [0,1]: the sums-matmul weight pairs.
    cstv = np.zeros((PART, 4), NP_BF16)
    cstv[:, 0] = 1
    cstv[:, 3] = 1

    in_maps = [
        {
            "qt": qt_all[core],
            "kt": kt_all[core],
            "vp": vp_all[core],
            "mb": mb_all[core],
            "cst": cstv,
        }
        for core in range(N_CORES)
    ]
    return perm, profile, in_maps


def kernel(queries, keys, values, valid_lens):
    q = np.ascontiguousarray(np.asarray(queries, dtype=np.float32))
    k = np.ascontiguousarray(np.asarray(keys, dtype=np.float32))
    v = np.ascontiguousarray(np.asarray(values, dtype=np.float32))
    lens = np.asarray(valid_lens).astype(np.int64).reshape(B)

    perm, profile, in_maps = host_prep(q, k, v, lens)

    if profile not in _NC_CACHE:
        _NC_CACHE[profile] = build_nc(profile)
    nc = _NC_CACHE[profile]

    res = run_bass_kernel_spmd(nc, in_maps, list(range(N_CORES)))

    out = np.empty((B, Q, D), np.float32)
    for core in range(N_CORES):
        core_out = res.results[core]["out"]    # [BPC, 128(v), 1024(q)]
        core_sums = res.results[core]["sums"]  # [BPC, 2, 512] -> flat [1024(q)]
        for slot in range(BPC):
            bidx = int(perm[slot, core])
            sums_flat = core_sums[slot].reshape(Q)
            out[bidx] = (
                core_out[slot].astype(np.float32) / sums_flat[None, :]
            ).T
    return out
